# revision 16
# baseline (speedup 1.0000x reference)
"""Bass/Trainium2 kernel for a 2-layer bidirectional QRNN (fo-pooling).

Reference computation (per layer, per direction):
    ZFO = X @ W + b            # [S, B, 3H]
    Z, F, O = split(ZFO); Z = tanh(Z); F = sigmoid(F); O = sigmoid(O)
    c_t = F_t * c_{t-1} + (1 - F_t) * Z_t        (bw direction: reversed time)
    Y_dir = O * C
    Y = concat(Y_fw, Y_bw)     # [S, B, 2H]
Two stacked layers; output is [B, S, 2H].

Sharding: data-parallel over batch. B=16 rows -> 2 rows per NeuronCore x 8.
Each core runs both layers for its 2 rows; no collectives.

v3 design (vs the fp32r/DRAM-y1 635us baseline):
- fp16 matmul operands, PRE-CAST ON THE HOST: X and all weights are fed
  to DRAM as float16, so matmul inputs DMA straight into SBUF with no
  on-chip casts and no staging at all (fp16, unlike fp32r, is a real
  DMA-able dtype). Measured matmul: 216ns/[128x128x512] (vs 233 fp32r --
  the 4-byte fp32r moving operand saturates SBUF read bandwidth).
  fp16 quantization of X/W adds ~1e-3 rel err (gate: 2e-2).
- The inter-layer activation Y1 lives entirely in SBUF as fp16 (8 chunk
  tiles [128, S] per row), never touching DRAM. Rows are processed
  b-major (L0fw, L0bw, L1fw, L1bw per row) so only one row's Y1 is live.
- Z-gate weight/bias columns are negated on the host, so the Tanh
  activation directly yields zneg = -z and the scan's g-input
  g = (1-f)*z = (f-1)*zneg is ONE fused DVE scalar_tensor_tensor.
- Post-PSUM values stay f32 (an fp16 scan measured SLOWER on DVE:
  1653ns vs 1455ns per [128,512]).
- Engine balance per layer-0 s-tile (PE: 10.4us): Scalar = 12
  activations (7.8); DVE = 4 scans + 4 fused g (9.9); GpSimd = 4 y-mults
  + carry columns (6.5). Input DMAs for s-tile i+1 issue at the START of
  s-tile i across both HWDGE queues.
- All weight DMAs are issued up front (w1b at L0bw start); they drain
  through queue slack long before first use. First matmul needs only the
  first w0f chunk + first input tile: ~3us after kernel start.

The time recurrence uses the DVE `tensor_tensor_scan` instruction
(state = f*state + g along the free axis); the bw direction runs the scan
through reversed access patterns with s-tiles processed in descending
order, chaining the carry via a [128,1] column copy.
"""

import numpy as np

import concourse.bacc as bacc
import concourse.mybir as mybir
from concourse import bass_utils
from concourse.tile import TileContext

# problem dims (hardcoded per spec)
B, S, D, H = 16, 2048, 512, 512
N_CORES = 8
BC = B // N_CORES  # batch rows per core
P = 128  # SBUF partitions
S_TILE = 512

F32 = mybir.dt.float32
FP16 = mybir.dt.float16
ACT = mybir.ActivationFunctionType
ALU = mybir.AluOpType


def build_nc(bc=BC, s=S, d=D, h=H, s_tile=S_TILE):
    """Build the SPMD Bass program (same program on every core)."""
    nc = bacc.Bacc("TRN2", target_bir_lowering=False)

    xt = nc.dram_tensor("xt", [bc, d, s], FP16, kind="ExternalInput")
    w0f = nc.dram_tensor("w0f", [d, 3 * h], FP16, kind="ExternalInput")
    w0b = nc.dram_tensor("w0b", [d, 3 * h], FP16, kind="ExternalInput")
    b0f = nc.dram_tensor("b0f", [P, 3 * (h // P)], F32, kind="ExternalInput")
    b0b = nc.dram_tensor("b0b", [P, 3 * (h // P)], F32, kind="ExternalInput")
    w1f = nc.dram_tensor("w1f", [2 * h, 3 * h], FP16, kind="ExternalInput")
    w1b = nc.dram_tensor("w1b", [2 * h, 3 * h], FP16, kind="ExternalInput")
    b1f = nc.dram_tensor("b1f", [P, 3 * (h // P)], F32, kind="ExternalInput")
    b1b = nc.dram_tensor("b1b", [P, 3 * (h // P)], F32, kind="ExternalInput")
    out_t = nc.dram_tensor("out_t", [bc, 2 * h, s], F32, kind="ExternalOutput")

    ns = s // s_tile
    hc = h // P
    k0 = d // P       # layer-0 contraction chunks
    k1 = 2 * h // P   # layer-1 contraction chunks

    with TileContext(nc) as tc:
        with (
            tc.tile_pool(name="wpool", bufs=1) as wpool,     # fp16 weights
            tc.tile_pool(name="y1pool", bufs=1) as y1pool,   # inter-layer act
            tc.tile_pool(name="scr", bufs=3) as spool,       # zn/f/o/g/c/y
            tc.tile_pool(name="carry", bufs=1) as cpool,
            tc.tile_pool(name="instream", bufs=1) as ypool,  # layer-0 input
            tc.tile_pool(name="ps", bufs=1, space="PSUM") as ppool,
        ):
            # ---------------- weights ----------------
            # One [P, k*3h] tile and ONE 3-D-AP DMA per weight set: DIRECT2D
            # issues cost ~0.7us of sequencer time each, so batching matters.
            wk = {
                nm: wpool.tile([P, kc * 3 * h], FP16, tag=nm, name=nm)
                for nm, kc in (("w0f", k0), ("w0b", k0),
                               ("w1f", k1), ("w1b", k1))
            }

            def wview(name, k, cols):
                base = k * 3 * h
                return wk[name][:, base + cols.start : base + cols.stop]

            def load_w(name, wd, k_chunks):
                src3 = wd[:, :].rearrange("(a p) c -> p a c", p=P)
                dst3 = wk[name][:].rearrange("p (a c) -> p a c", a=k_chunks)
                nc.sync.dma_start(dst3, src3)

            def load_biases(bd, prefix):
                """One plain DMA of the host-pretabled [P, 3*hc] bias;
                returns per-(gate, h-chunk) [P, 1] views."""
                btab = wpool.tile([P, 3 * hc], F32, tag=f"{prefix}_btab",
                                  name=f"{prefix}_btab")
                nc.sync.dma_start(btab[:], bd[:, :])
                return {
                    (g, hh): btab[:, g * hc + hh : g * hc + hh + 1]
                    for g in range(3)
                    for hh in range(hc)
                }

            # PE warm-up: dummy matmuls on memset scratch keep the PE
            # busy through the DMA bring-up window, so the p-state governor
            # is at full clock when the first real matmul issues.
            warm_w = wpool.tile([P, P], FP16, tag="warm_w", name="warm_w")
            warm_m = wpool.tile([P, s_tile], FP16, tag="warm_m", name="warm_m")
            nc.gpsimd.memset(warm_w[:], 0.0)
            nc.gpsimd.memset(warm_m[:], 0.0)
            warm_ps = ppool.tile([P, s_tile], F32, tag="ps2", name="warm_ps",
                                 bufs=2)
            for _ in range(13):
                nc.tensor.matmul(warm_ps[:], warm_w[:], warm_m[:],
                                 start=True, stop=True)
            warm_act = wpool.tile([P, 1], F32, tag="warm_act", name="warm_act")
            nc.scalar.activation(warm_act[:], warm_w[:, 0:1], ACT.Sigmoid)
            nc.scalar.activation(warm_act[:], warm_w[:, 0:1], ACT.Tanh)

            bt = {}
            # Queue order at startup: w0f first (gates the first matmul; two
            # half-DMAs so chunk 0 lands early), then the b0f bias (gates
            # the first act), then input tile 0.
            half = k0 // 2 * P
            for lo, hi in ((0, half), (half, d)):
                src3 = w0f[lo:hi, :].rearrange("(a p) c -> p a c", p=P)
                dst3 = wk["w0f"][:, lo * 3 * h // P : hi * 3 * h // P].rearrange(
                    "p (a c) -> p a c", a=(hi - lo) // P)
                nc.sync.dma_start(dst3, src3)
            # ---------------- layer-0 input prefetch ----------------
            l0_iters = []
            for b in range(bc):
                for fw in (True, False):
                    order = range(ns) if fw else range(ns - 1, -1, -1)
                    for si in order:
                        l0_iters.append((b, fw, si))
            l0_pos = {key: i for i, key in enumerate(l0_iters)}
            in_tiles = {}

            def issue_l0_input(key, q=None):
                b, fw, si = key
                s0 = si * s_tile
                t = ypool.tile([P, k0 * s_tile], FP16, tag="inr", bufs=2,
                               name="inr")
                src3 = xt[b, :, s0 : s0 + s_tile].rearrange(
                    "(k p) s -> p k s", p=P)
                (q or nc.sync).dma_start(
                    t[:].rearrange("p (k s) -> p k s", k=k0), src3)
                in_tiles[key] = t

            issue_l0_input(l0_iters[0], q=nc.scalar)
            btab0f = wpool.tile([P, 3 * hc], F32, tag="b0f_btab",
                                name="b0f_btab")
            nc.scalar.dma_start(btab0f[:], b0f[:, :])
            bt["w0f"] = {
                (g, hh): btab0f[:, g * hc + hh : g * hc + hh + 1]
                for g in range(3) for hh in range(hc)
            }
            # Remaining weights: queue now, they drain through slack long
            # before first use (w0b ~45us in, w1f ~90us in).
            bt["w0b"] = load_biases(b0b, "b0b")
            bt["w1f"] = load_biases(b1f, "b1f")
            load_w("w0b", w0b, k0)
            load_w("w1f", w1f, k1)

            y1c = {}  # chunk index 0..7 -> current row's SBUF tile

            for b in range(bc):
                for layer in (0, 1):
                    for fw in ((True, False) if layer == 0 else (False, True)):
                        wname = f"w{layer}{'f' if fw else 'b'}"
                        kch = k0 if layer == 0 else k1
                        dir_off = 0 if fw else h
                        s_order = (list(range(ns)) if fw
                                   else list(range(ns - 1, -1, -1)))
                        if b == 0 and layer == 0 and not fw:
                            bt["w1b"] = load_biases(b1b, "b1b")
                            load_w("w1b", w1b, k1)
                        btile = bt[wname]
                        carry = [cpool.tile([P, 1], F32, tag=f"c{hh}",
                                            name=f"carry{hh}")
                                 for hh in range(hc)]
                        if layer == 0:
                            off = 0 if fw else hc
                            for c in range(hc):
                                y1c[off + c] = y1pool.tile(
                                    [P, s], FP16, tag=f"y1c{off + c}",
                                    name=f"y1c{off + c}")
                        for si, s_idx in enumerate(s_order):
                            s0 = s_idx * s_tile
                            if layer == 0:
                                pos = l0_pos[(b, fw, s_idx)]
                                if pos + 1 < len(l0_iters):
                                    issue_l0_input(l0_iters[pos + 1])
                                t = in_tiles.pop((b, fw, s_idx))
                                ins = [t[:, k * s_tile : (k + 1) * s_tile]
                                       for k in range(k0)]
                            else:
                                ins = [y1c[k][:, s0 : s0 + s_tile]
                                       for k in range(k1)]
                            for hh in range(hc):
                                ps = [
                                    ppool.tile([P, s_tile], F32, tag=f"ps{g}",
                                               name=f"ps{g}",
                                               bufs=(3 if g < 2 else 2))
                                    for g in range(3)
                                ]
                                for g in range(3):
                                    cols = slice(g * h + hh * P,
                                                 g * h + (hh + 1) * P)
                                    for k in range(kch):
                                        nc.tensor.matmul(
                                            ps[g][:],
                                            wview(wname, k, cols),
                                            ins[k],
                                            start=(k == 0),
                                            stop=(k == kch - 1),
                                        )
                                zn = spool.tile([P, s_tile], F32, tag="zn",
                                                name="zn")
                                f_ = spool.tile([P, s_tile], F32, tag="f",
                                                name="f", bufs=4)
                                o = spool.tile([P, s_tile], F32, tag="o",
                                               name="o", bufs=6)
                                g_ = spool.tile([P, s_tile], F32, tag="g",
                                                name="g")
                                c_ = spool.tile([P, s_tile], F32, tag="c",
                                                name="c", bufs=6)
                                nc.scalar.activation(f_[:], ps[1][:],
                                                     ACT.Sigmoid,
                                                     bias=btile[1, hh][:])
                                # host negated the z-gate W/b: Tanh gives -z
                                nc.scalar.activation(zn[:], ps[0][:], ACT.Tanh,
                                                     bias=btile[0, hh][:])
                                nc.scalar.activation(o[:], ps[2][:],
                                                     ACT.Sigmoid,
                                                     bias=btile[2, hh][:])
                                # g = (f - 1) * (-z) = (1 - f) z, one DVE op
                                nc.vector.scalar_tensor_tensor(
                                    g_[:], f_[:], 1.0, zn[:],
                                    ALU.subtract, ALU.mult)
                                # c_t = f_t * c_prev + g_t (bw: reversed time)
                                if fw:
                                    sc = (c_[:], f_[:], g_[:])
                                    carry_col = slice(s_tile - 1, s_tile)
                                else:
                                    sc = (c_[:, ::-1], f_[:, ::-1], g_[:, ::-1])
                                    carry_col = slice(0, 1)
                                init = 0.0 if si == 0 else carry[hh][:]
                                nc.vector.tensor_tensor_scan(
                                    sc[0], sc[1], sc[2], init,
                                    ALU.mult, ALU.add)
                                if si < ns - 1:
                                    nc.gpsimd.tensor_copy(carry[hh][:],
                                                          c_[:, carry_col])
                                if layer == 0:
                                    chunk = y1c[(0 if fw else hc) + hh]
                                    nc.gpsimd.tensor_mul(
                                        chunk[:, s0 : s0 + s_tile],
                                        o[:], c_[:])
                                else:
                                    last_it = (b == bc - 1 and fw
                                               and si == ns - 1)
                                    if hh == 0:
                                        ybig = spool.tile(
                                            [P, hc * s_tile], F32, tag="y",
                                            name="y", bufs=2)
                                    ymul = (nc.vector.tensor_mul if last_it
                                            else nc.gpsimd.tensor_mul)
                                    ymul(
                                        ybig[:, hh * s_tile : (hh + 1) * s_tile],
                                        o[:], c_[:])
                                    flush = ((hh == hc - 1) if not last_it
                                             else (hh % 2 == 1))
                                    if flush:
                                        a0 = 0 if (not last_it or hh == 1) \
                                            else 2
                                        r0 = dir_off + a0 * P
                                        na = hh + 1 - a0
                                        dst3 = out_t[
                                            b, r0 : r0 + na * P,
                                            s0 : s0 + s_tile].rearrange(
                                                "(a p) c -> p a c", p=P)
                                        nc.sync.dma_start(
                                            dst3,
                                            ybig[:, a0 * s_tile :
                                                 (hh + 1) * s_tile].rearrange(
                                                "p (a c) -> p a c", a=na))

    nc.finalize()
    return nc


_NC_CACHE = {}


def _get_nc():
    if "v3" not in _NC_CACHE:
        _NC_CACHE["v3"] = build_nc()
    return _NC_CACHE["v3"]


def kernel(X, seqlens, W_fw0, b_fw0, W_bw0, b_bw0, W_fw1, b_fw1, W_bw1, b_bw1,
           mm_dtype="fp16", trace=False):
    """Full-input entry point: shards over 8 cores, returns [B, S, 2H] f32."""
    del seqlens, mm_dtype  # seqlens unused by the reference computation
    X = np.asarray(X, dtype=np.float32)

    def neg_z(v, dt):
        """Negate the z-gate block so tanh(ps+b) = -z on device."""
        v = np.array(np.asarray(v, dtype=np.float32))
        v[..., :H] *= -1.0
        return np.ascontiguousarray(v.astype(dt))

    def btab(v):
        """[3H] bias -> [P, 3*H/P] column table (z-gate negated)."""
        v = neg_z(v, np.float32)  # [3H]
        t = v.reshape(3 * H // P, P).T  # [P, 3hc]
        return np.ascontiguousarray(t)

    weights = {
        "w0f": neg_z(W_fw0, np.float16), "b0f": btab(b_fw0),
        "w0b": neg_z(W_bw0, np.float16), "b0b": btab(b_bw0),
        "w1f": neg_z(W_fw1, np.float16), "b1f": btab(b_fw1),
        "w1b": neg_z(W_bw1, np.float16), "b1b": btab(b_bw1),
    }

    nc = _get_nc()
    in_maps = []
    for i in range(N_CORES):
        rows = X[i * BC : (i + 1) * BC]  # [BC, S, D]
        xt_i = np.ascontiguousarray(
            rows.transpose(0, 2, 1).astype(np.float16))  # [BC, D, S] fp16
        in_maps.append({"xt": xt_i, **weights})

    res = bass_utils.run_bass_kernel_spmd(
        nc, in_maps, core_ids=list(range(N_CORES)), trace=trace
    )
    out = np.empty((B, S, 2 * H), dtype=np.float32)
    for i in range(N_CORES):
        out_t = res.results[i]["out_t"]  # [BC, 2H, S]
        out[i * BC : (i + 1) * BC] = out_t.transpose(0, 2, 1)
    kernel.last_results = res
    return out


# revision 17
# speedup vs baseline: 1.0049x; 1.0049x over previous
"""Bass/Trainium2 kernel for a 2-layer bidirectional QRNN (fo-pooling).

Reference computation (per layer, per direction):
    ZFO = X @ W + b            # [S, B, 3H]
    Z, F, O = split(ZFO); Z = tanh(Z); F = sigmoid(F); O = sigmoid(O)
    c_t = F_t * c_{t-1} + (1 - F_t) * Z_t        (bw direction: reversed time)
    Y_dir = O * C
    Y = concat(Y_fw, Y_bw)     # [S, B, 2H]
Two stacked layers; output is [B, S, 2H].

Sharding: data-parallel over batch. B=16 rows -> 2 rows per NeuronCore x 8.
Each core runs both layers for its 2 rows; no collectives.

v3 design (vs the fp32r/DRAM-y1 635us baseline):
- fp16 matmul operands, PRE-CAST ON THE HOST: X and all weights are fed
  to DRAM as float16, so matmul inputs DMA straight into SBUF with no
  on-chip casts and no staging at all (fp16, unlike fp32r, is a real
  DMA-able dtype). Measured matmul: 216ns/[128x128x512] (vs 233 fp32r --
  the 4-byte fp32r moving operand saturates SBUF read bandwidth).
  fp16 quantization of X/W adds ~1e-3 rel err (gate: 2e-2).
- The inter-layer activation Y1 lives entirely in SBUF as fp16 (8 chunk
  tiles [128, S] per row), never touching DRAM. Rows are processed
  b-major (L0fw, L0bw, L1fw, L1bw per row) so only one row's Y1 is live.
- Z-gate weight/bias columns are negated on the host, so the Tanh
  activation directly yields zneg = -z and the scan's g-input
  g = (1-f)*z = (f-1)*zneg is ONE fused DVE scalar_tensor_tensor.
- Post-PSUM values stay f32 (an fp16 scan measured SLOWER on DVE:
  1653ns vs 1455ns per [128,512]).
- Engine balance per layer-0 s-tile (PE: 10.4us): Scalar = 12
  activations (7.8); DVE = 4 scans + 4 fused g (9.9); GpSimd = 4 y-mults
  + carry columns (6.5). Input DMAs for s-tile i+1 issue at the START of
  s-tile i across both HWDGE queues.
- All weight DMAs are issued up front (w1b at L0bw start); they drain
  through queue slack long before first use. First matmul needs only the
  first w0f chunk + first input tile: ~3us after kernel start.

The time recurrence uses the DVE `tensor_tensor_scan` instruction
(state = f*state + g along the free axis); the bw direction runs the scan
through reversed access patterns with s-tiles processed in descending
order, chaining the carry via a [128,1] column copy.
"""

import numpy as np

import concourse.bacc as bacc
import concourse.mybir as mybir
from concourse import bass_utils
from concourse.tile import TileContext

# problem dims (hardcoded per spec)
B, S, D, H = 16, 2048, 512, 512
N_CORES = 8
BC = B // N_CORES  # batch rows per core
P = 128  # SBUF partitions
S_TILE = 512

F32 = mybir.dt.float32
FP16 = mybir.dt.float16
ACT = mybir.ActivationFunctionType
ALU = mybir.AluOpType


def build_nc(bc=BC, s=S, d=D, h=H, s_tile=S_TILE):
    """Build the SPMD Bass program (same program on every core)."""
    nc = bacc.Bacc("TRN2", target_bir_lowering=False)

    xt = nc.dram_tensor("xt", [bc, d, s], FP16, kind="ExternalInput")
    w0f = nc.dram_tensor("w0f", [d, 3 * h], FP16, kind="ExternalInput")
    w0b = nc.dram_tensor("w0b", [d, 3 * h], FP16, kind="ExternalInput")
    b0f = nc.dram_tensor("b0f", [P, 3 * (h // P)], F32, kind="ExternalInput")
    b0b = nc.dram_tensor("b0b", [P, 3 * (h // P)], F32, kind="ExternalInput")
    w1f = nc.dram_tensor("w1f", [2 * h, 3 * h], FP16, kind="ExternalInput")
    w1b = nc.dram_tensor("w1b", [2 * h, 3 * h], FP16, kind="ExternalInput")
    b1f = nc.dram_tensor("b1f", [P, 3 * (h // P)], F32, kind="ExternalInput")
    b1b = nc.dram_tensor("b1b", [P, 3 * (h // P)], F32, kind="ExternalInput")
    out_t = nc.dram_tensor("out_t", [bc, 2 * h, s], F32, kind="ExternalOutput")

    ns = s // s_tile
    hc = h // P
    k0 = d // P       # layer-0 contraction chunks
    k1 = 2 * h // P   # layer-1 contraction chunks

    with TileContext(nc) as tc:
        with (
            tc.tile_pool(name="wpool", bufs=1) as wpool,     # fp16 weights
            tc.tile_pool(name="y1pool", bufs=1) as y1pool,   # inter-layer act
            tc.tile_pool(name="scr", bufs=3) as spool,       # zn/f/o/g/c/y
            tc.tile_pool(name="carry", bufs=1) as cpool,
            tc.tile_pool(name="instream", bufs=1) as ypool,  # layer-0 input
            tc.tile_pool(name="ps", bufs=1, space="PSUM") as ppool,
        ):
            # ---------------- weights ----------------
            # One [P, k*3h] tile and ONE 3-D-AP DMA per weight set: DIRECT2D
            # issues cost ~0.7us of sequencer time each, so batching matters.
            wk = {
                nm: wpool.tile([P, kc * 3 * h], FP16, tag=nm, name=nm)
                for nm, kc in (("w0f", k0), ("w0b", k0),
                               ("w1f", k1), ("w1b", k1))
            }

            def wview(name, k, cols):
                base = k * 3 * h
                return wk[name][:, base + cols.start : base + cols.stop]

            def load_w(name, wd, k_chunks):
                src3 = wd[:, :].rearrange("(a p) c -> p a c", p=P)
                dst3 = wk[name][:].rearrange("p (a c) -> p a c", a=k_chunks)
                nc.sync.dma_start(dst3, src3)

            def load_biases(bd, prefix):
                """One plain DMA of the host-pretabled [P, 3*hc] bias;
                returns per-(gate, h-chunk) [P, 1] views."""
                btab = wpool.tile([P, 3 * hc], F32, tag=f"{prefix}_btab",
                                  name=f"{prefix}_btab")
                nc.sync.dma_start(btab[:], bd[:, :])
                return {
                    (g, hh): btab[:, g * hc + hh : g * hc + hh + 1]
                    for g in range(3)
                    for hh in range(hc)
                }

            # PE warm-up: dummy matmuls on memset scratch keep the PE
            # busy through the DMA bring-up window, so the p-state governor
            # is at full clock when the first real matmul issues.
            warm_w = wpool.tile([P, P], FP16, tag="warm_w", name="warm_w")
            warm_m = wpool.tile([P, s_tile], FP16, tag="warm_m", name="warm_m")
            nc.gpsimd.memset(warm_w[:], 0.0)
            nc.gpsimd.memset(warm_m[:], 0.0)
            warm_ps = ppool.tile([P, s_tile], F32, tag="ps2", name="warm_ps",
                                 bufs=2)
            for _ in range(13):
                nc.tensor.matmul(warm_ps[:], warm_w[:], warm_m[:],
                                 start=True, stop=True)
            warm_act = wpool.tile([P, 1], F32, tag="warm_act", name="warm_act")
            nc.scalar.activation(warm_act[:], warm_w[:, 0:1], ACT.Sigmoid)
            nc.scalar.activation(warm_act[:], warm_w[:, 0:1], ACT.Tanh)

            bt = {}
            # Queue order at startup: w0f first (gates the first matmul),
            # then the b0f bias (gates the first act), then input tile 0.
            load_w("w0f", w0f, k0)
            bt["w0f"] = load_biases(b0f, "b0f")
            # ---------------- layer-0 input prefetch ----------------
            l0_iters = []
            for b in range(bc):
                for fw in (True, False):
                    order = range(ns) if fw else range(ns - 1, -1, -1)
                    for si in order:
                        l0_iters.append((b, fw, si))
            l0_pos = {key: i for i, key in enumerate(l0_iters)}
            in_tiles = {}

            def issue_l0_input(key):
                b, fw, si = key
                s0 = si * s_tile
                t = ypool.tile([P, k0 * s_tile], FP16, tag="inr", bufs=2,
                               name="inr")
                src3 = xt[b, :, s0 : s0 + s_tile].rearrange(
                    "(k p) s -> p k s", p=P)
                nc.sync.dma_start(
                    t[:].rearrange("p (k s) -> p k s", k=k0), src3)
                in_tiles[key] = t

            issue_l0_input(l0_iters[0])
            # Remaining weights: queue now, they drain through slack long
            # before first use (w0b ~45us in, w1f ~90us in).
            bt["w0b"] = load_biases(b0b, "b0b")
            bt["w1f"] = load_biases(b1f, "b1f")
            load_w("w0b", w0b, k0)
            load_w("w1f", w1f, k1)

            y1c = {}  # chunk index 0..7 -> current row's SBUF tile

            for b in range(bc):
                for layer in (0, 1):
                    for fw in ((True, False) if layer == 0 else (False, True)):
                        wname = f"w{layer}{'f' if fw else 'b'}"
                        kch = k0 if layer == 0 else k1
                        dir_off = 0 if fw else h
                        s_order = (list(range(ns)) if fw
                                   else list(range(ns - 1, -1, -1)))
                        if b == 0 and layer == 0 and not fw:
                            bt["w1b"] = load_biases(b1b, "b1b")
                            load_w("w1b", w1b, k1)
                        btile = bt[wname]
                        carry = [cpool.tile([P, 1], F32, tag=f"c{hh}",
                                            name=f"carry{hh}")
                                 for hh in range(hc)]
                        if layer == 0:
                            off = 0 if fw else hc
                            for c in range(hc):
                                y1c[off + c] = y1pool.tile(
                                    [P, s], FP16, tag=f"y1c{off + c}",
                                    name=f"y1c{off + c}")
                        for si, s_idx in enumerate(s_order):
                            s0 = s_idx * s_tile
                            if layer == 0:
                                pos = l0_pos[(b, fw, s_idx)]
                                if pos + 1 < len(l0_iters):
                                    issue_l0_input(l0_iters[pos + 1])
                                t = in_tiles.pop((b, fw, s_idx))
                                ins = [t[:, k * s_tile : (k + 1) * s_tile]
                                       for k in range(k0)]
                            else:
                                ins = [y1c[k][:, s0 : s0 + s_tile]
                                       for k in range(k1)]
                            for hh in range(hc):
                                ps = [
                                    ppool.tile([P, s_tile], F32, tag=f"ps{g}",
                                               name=f"ps{g}",
                                               bufs=(3 if g < 2 else 2))
                                    for g in range(3)
                                ]
                                for g in range(3):
                                    cols = slice(g * h + hh * P,
                                                 g * h + (hh + 1) * P)
                                    for k in range(kch):
                                        nc.tensor.matmul(
                                            ps[g][:],
                                            wview(wname, k, cols),
                                            ins[k],
                                            start=(k == 0),
                                            stop=(k == kch - 1),
                                        )
                                zn = spool.tile([P, s_tile], F32, tag="zn",
                                                name="zn")
                                f_ = spool.tile([P, s_tile], F32, tag="f",
                                                name="f", bufs=4)
                                o = spool.tile([P, s_tile], F32, tag="o",
                                               name="o", bufs=6)
                                g_ = spool.tile([P, s_tile], F32, tag="g",
                                                name="g")
                                c_ = spool.tile([P, s_tile], F32, tag="c",
                                                name="c", bufs=6)
                                nc.scalar.activation(f_[:], ps[1][:],
                                                     ACT.Sigmoid,
                                                     bias=btile[1, hh][:])
                                # host negated the z-gate W/b: Tanh gives -z
                                nc.scalar.activation(zn[:], ps[0][:], ACT.Tanh,
                                                     bias=btile[0, hh][:])
                                nc.scalar.activation(o[:], ps[2][:],
                                                     ACT.Sigmoid,
                                                     bias=btile[2, hh][:])
                                # g = (f - 1) * (-z) = (1 - f) z, one DVE op
                                nc.vector.scalar_tensor_tensor(
                                    g_[:], f_[:], 1.0, zn[:],
                                    ALU.subtract, ALU.mult)
                                # c_t = f_t * c_prev + g_t (bw: reversed time)
                                if fw:
                                    sc = (c_[:], f_[:], g_[:])
                                    carry_col = slice(s_tile - 1, s_tile)
                                else:
                                    sc = (c_[:, ::-1], f_[:, ::-1], g_[:, ::-1])
                                    carry_col = slice(0, 1)
                                init = 0.0 if si == 0 else carry[hh][:]
                                nc.vector.tensor_tensor_scan(
                                    sc[0], sc[1], sc[2], init,
                                    ALU.mult, ALU.add)
                                if si < ns - 1:
                                    nc.gpsimd.tensor_copy(carry[hh][:],
                                                          c_[:, carry_col])
                                if layer == 0:
                                    chunk = y1c[(0 if fw else hc) + hh]
                                    nc.gpsimd.tensor_mul(
                                        chunk[:, s0 : s0 + s_tile],
                                        o[:], c_[:])
                                else:
                                    if hh == 0:
                                        ybig = spool.tile(
                                            [P, hc * s_tile], F32, tag="y",
                                            name="y", bufs=2)
                                    nc.gpsimd.tensor_mul(
                                        ybig[:, hh * s_tile : (hh + 1) * s_tile],
                                        o[:], c_[:])
                                    if hh == hc - 1:
                                        dst3 = out_t[
                                            b, dir_off : dir_off + h,
                                            s0 : s0 + s_tile].rearrange(
                                                "(a p) c -> p a c", p=P)
                                        nc.sync.dma_start(
                                            dst3,
                                            ybig[:].rearrange(
                                                "p (a c) -> p a c", a=hc))

    nc.finalize()
    return nc


_NC_CACHE = {}


def _get_nc():
    if "v3" not in _NC_CACHE:
        _NC_CACHE["v3"] = build_nc()
    return _NC_CACHE["v3"]


def kernel(X, seqlens, W_fw0, b_fw0, W_bw0, b_bw0, W_fw1, b_fw1, W_bw1, b_bw1,
           mm_dtype="fp16", trace=False):
    """Full-input entry point: shards over 8 cores, returns [B, S, 2H] f32."""
    del seqlens, mm_dtype  # seqlens unused by the reference computation
    X = np.asarray(X, dtype=np.float32)

    def neg_z(v, dt):
        """Negate the z-gate block so tanh(ps+b) = -z on device."""
        v = np.array(np.asarray(v, dtype=np.float32))
        v[..., :H] *= -1.0
        return np.ascontiguousarray(v.astype(dt))

    def btab(v):
        """[3H] bias -> [P, 3*H/P] column table (z-gate negated)."""
        v = neg_z(v, np.float32)  # [3H]
        t = v.reshape(3 * H // P, P).T  # [P, 3hc]
        return np.ascontiguousarray(t)

    weights = {
        "w0f": neg_z(W_fw0, np.float16), "b0f": btab(b_fw0),
        "w0b": neg_z(W_bw0, np.float16), "b0b": btab(b_bw0),
        "w1f": neg_z(W_fw1, np.float16), "b1f": btab(b_fw1),
        "w1b": neg_z(W_bw1, np.float16), "b1b": btab(b_bw1),
    }

    nc = _get_nc()
    in_maps = []
    for i in range(N_CORES):
        rows = X[i * BC : (i + 1) * BC]  # [BC, S, D]
        xt_i = np.ascontiguousarray(
            rows.transpose(0, 2, 1).astype(np.float16))  # [BC, D, S] fp16
        in_maps.append({"xt": xt_i, **weights})

    res = bass_utils.run_bass_kernel_spmd(
        nc, in_maps, core_ids=list(range(N_CORES)), trace=trace
    )
    out = np.empty((B, S, 2 * H), dtype=np.float32)
    for i in range(N_CORES):
        out_t = res.results[i]["out_t"]  # [BC, 2H, S]
        out[i * BC : (i + 1) * BC] = out_t.transpose(0, 2, 1)
    kernel.last_results = res
    return out


# revision 18
# speedup vs baseline: 1.0093x; 1.0044x over previous
"""Bass/Trainium2 kernel for a 2-layer bidirectional QRNN (fo-pooling).

Reference computation (per layer, per direction):
    ZFO = X @ W + b            # [S, B, 3H]
    Z, F, O = split(ZFO); Z = tanh(Z); F = sigmoid(F); O = sigmoid(O)
    c_t = F_t * c_{t-1} + (1 - F_t) * Z_t        (bw direction: reversed time)
    Y_dir = O * C
    Y = concat(Y_fw, Y_bw)     # [S, B, 2H]
Two stacked layers; output is [B, S, 2H].

Sharding: data-parallel over batch. B=16 rows -> 2 rows per NeuronCore x 8.
Each core runs both layers for its 2 rows; no collectives.

Design (evolved from a 635us fp32r/DRAM-roundtrip baseline; now ~525us,
PE-bound at ~96% of the 2304-matmul fp16 streaming floor):
- fp16 matmul operands, PRE-CAST ON THE HOST: X and all weights are fed
  to DRAM as float16, so matmul inputs DMA straight into SBUF with no
  on-chip casts or staging (fp16, unlike fp32r, is a real DMA-able
  dtype). Measured matmul: 216ns/[128x128x512] vs 233ns fp32r (the
  4-byte fp32r moving operand saturates SBUF read bandwidth). fp16
  quantization of X/W adds ~5e-4 rel err (gate: 2e-2).
- The inter-layer activation Y1 lives entirely in SBUF as fp16 (8 chunk
  tiles [128, S] per row), never touching DRAM. Rows are processed
  b-major (L0fw, L0bw, L1bw, L1fw per row) so only one row's Y1 is
  live. Layer 1 runs bw FIRST: its first input tile (s = S-tile) is
  finished at L0bw's first iteration, while L1fw's first input (s = 0)
  is only finished by L0bw's LAST iteration -- bw-first removes an
  ~8.6us/row PE stall at the layer boundary.
- Z-gate weight/bias columns are negated on the host, so the Tanh
  activation directly yields zneg = -z and the scan's g-input
  g = (1-f)*z = (f-1)*zneg is ONE fused DVE scalar_tensor_tensor
  (scalar_tensor_tensor is NOT supported on GpSimd). Biases are
  host-pretabled to [P, 3*H/P] so their DMA is contiguous (a
  rearranging gather DMA cost ~8us of descriptor processing).
- Post-PSUM values stay f32 (an fp16 scan measured SLOWER on DVE).
- Engine balance per layer-0 s-tile (PE: 10.4us): Scalar = 12
  activations (7.8); DVE = 4 scans + 4 fused g (9.6 incl slot waits);
  GpSimd = 4 y-mults + carry columns (6.5).
- DMA discipline: a DIRECT2D issue costs ~0.7us of SEQUENCER time, so
  (a) every multi-part transfer is batched into one 3-D-AP DMA (one per
  weight set, one per input s-tile, one per output s-tile), and (b) all
  DMAs ride the sync queue, keeping the Scalar sequencer free to decode
  activations (DMA issues queued ahead of acts once stalled the act
  stream ~10us and starved PSUM recycling). Input s-tile i+1 is
  prefetched at the start of s-tile i.
- Cold-start: 13 dummy matmuls on memset scratch hold the PE p-state at
  full clock through the ~5us DMA bring-up (cold matmuls run 2-2.7x
  slow), and two dummy activations force both act-table loads into the
  preamble window (an inline table switch costs 1.3us mid-pipeline).

The time recurrence uses the DVE `tensor_tensor_scan` instruction
(state = f*state + g along the free axis); the bw direction runs the scan
through reversed access patterns with s-tiles processed in descending
order, chaining the carry via a [128,1] column copy.
"""

import numpy as np

import concourse.bacc as bacc
import concourse.mybir as mybir
from concourse import bass_utils
from concourse.tile import TileContext

# problem dims (hardcoded per spec)
B, S, D, H = 16, 2048, 512, 512
N_CORES = 8
BC = B // N_CORES  # batch rows per core
P = 128  # SBUF partitions
S_TILE = 512

F32 = mybir.dt.float32
FP16 = mybir.dt.float16
ACT = mybir.ActivationFunctionType
ALU = mybir.AluOpType


def build_nc(bc=BC, s=S, d=D, h=H, s_tile=S_TILE):
    """Build the SPMD Bass program (same program on every core)."""
    nc = bacc.Bacc("TRN2", target_bir_lowering=False)

    xt = nc.dram_tensor("xt", [bc, d, s], FP16, kind="ExternalInput")
    w0f = nc.dram_tensor("w0f", [d, 3 * h], FP16, kind="ExternalInput")
    w0b = nc.dram_tensor("w0b", [d, 3 * h], FP16, kind="ExternalInput")
    b0f = nc.dram_tensor("b0f", [P, 3 * (h // P)], F32, kind="ExternalInput")
    b0b = nc.dram_tensor("b0b", [P, 3 * (h // P)], F32, kind="ExternalInput")
    w1f = nc.dram_tensor("w1f", [2 * h, 3 * h], FP16, kind="ExternalInput")
    w1b = nc.dram_tensor("w1b", [2 * h, 3 * h], FP16, kind="ExternalInput")
    b1f = nc.dram_tensor("b1f", [P, 3 * (h // P)], F32, kind="ExternalInput")
    b1b = nc.dram_tensor("b1b", [P, 3 * (h // P)], F32, kind="ExternalInput")
    out_t = nc.dram_tensor("out_t", [bc, 2 * h, s], F32, kind="ExternalOutput")

    ns = s // s_tile
    hc = h // P
    k0 = d // P       # layer-0 contraction chunks
    k1 = 2 * h // P   # layer-1 contraction chunks

    with TileContext(nc) as tc:
        with (
            tc.tile_pool(name="wpool", bufs=1) as wpool,     # fp16 weights
            tc.tile_pool(name="y1pool", bufs=1) as y1pool,   # inter-layer act
            tc.tile_pool(name="scr", bufs=3) as spool,       # zn/f/o/g/c/y
            tc.tile_pool(name="carry", bufs=1) as cpool,
            tc.tile_pool(name="instream", bufs=1) as ypool,  # layer-0 input
            tc.tile_pool(name="ps", bufs=1, space="PSUM") as ppool,
        ):
            # ---------------- weights ----------------
            # One [P, k*3h] tile and ONE 3-D-AP DMA per weight set: DIRECT2D
            # issues cost ~0.7us of sequencer time each, so batching matters.
            wk = {
                nm: wpool.tile([P, kc * 3 * h], FP16, tag=nm, name=nm)
                for nm, kc in (("w0f", k0), ("w0b", k0),
                               ("w1f", k1), ("w1b", k1))
            }

            def wview(name, k, cols):
                base = k * 3 * h
                return wk[name][:, base + cols.start : base + cols.stop]

            def load_w(name, wd, k_chunks):
                src3 = wd[:, :].rearrange("(a p) c -> p a c", p=P)
                dst3 = wk[name][:].rearrange("p (a c) -> p a c", a=k_chunks)
                nc.sync.dma_start(dst3, src3)

            def load_biases(bd, prefix):
                """One plain DMA of the host-pretabled [P, 3*hc] bias;
                returns per-(gate, h-chunk) [P, 1] views."""
                btab = wpool.tile([P, 3 * hc], F32, tag=f"{prefix}_btab",
                                  name=f"{prefix}_btab")
                nc.sync.dma_start(btab[:], bd[:, :])
                return {
                    (g, hh): btab[:, g * hc + hh : g * hc + hh + 1]
                    for g in range(3)
                    for hh in range(hc)
                }

            # PE warm-up: dummy matmuls on memset scratch keep the PE
            # busy through the DMA bring-up window, so the p-state governor
            # is at full clock when the first real matmul issues.
            warm_w = wpool.tile([P, P], FP16, tag="warm_w", name="warm_w")
            warm_m = wpool.tile([P, s_tile], FP16, tag="warm_m", name="warm_m")
            nc.gpsimd.memset(warm_w[:], 0.0)
            nc.gpsimd.memset(warm_m[:], 0.0)
            warm_ps = ppool.tile([P, s_tile], F32, tag="ps2", name="warm_ps",
                                 bufs=2)
            for _ in range(13):
                nc.tensor.matmul(warm_ps[:], warm_w[:], warm_m[:],
                                 start=True, stop=True)
            warm_act = wpool.tile([P, 1], F32, tag="warm_act", name="warm_act")
            nc.scalar.activation(warm_act[:], warm_w[:, 0:1], ACT.Sigmoid)
            nc.scalar.activation(warm_act[:], warm_w[:, 0:1], ACT.Tanh)

            bt = {}
            # Queue order at startup: w0f first (gates the first matmul),
            # then the b0f bias (gates the first act), then input tile 0.
            load_w("w0f", w0f, k0)
            bt["w0f"] = load_biases(b0f, "b0f")
            # ---------------- layer-0 input prefetch ----------------
            l0_iters = []
            for b in range(bc):
                for fw in (True, False):
                    order = range(ns) if fw else range(ns - 1, -1, -1)
                    for si in order:
                        l0_iters.append((b, fw, si))
            l0_pos = {key: i for i, key in enumerate(l0_iters)}
            in_tiles = {}

            def issue_l0_input(key):
                b, fw, si = key
                s0 = si * s_tile
                t = ypool.tile([P, k0 * s_tile], FP16, tag="inr", bufs=2,
                               name="inr")
                src3 = xt[b, :, s0 : s0 + s_tile].rearrange(
                    "(k p) s -> p k s", p=P)
                nc.sync.dma_start(
                    t[:].rearrange("p (k s) -> p k s", k=k0), src3)
                in_tiles[key] = t

            issue_l0_input(l0_iters[0])
            # Remaining weights: queue now, they drain through slack long
            # before first use (w0b ~45us in, w1f ~90us in).
            bt["w0b"] = load_biases(b0b, "b0b")
            bt["w1f"] = load_biases(b1f, "b1f")
            load_w("w0b", w0b, k0)
            load_w("w1f", w1f, k1)

            y1c = {}  # chunk index 0..7 -> current row's SBUF tile

            for b in range(bc):
                for layer in (0, 1):
                    for fw in ((True, False) if layer == 0 else (False, True)):
                        wname = f"w{layer}{'f' if fw else 'b'}"
                        kch = k0 if layer == 0 else k1
                        dir_off = 0 if fw else h
                        s_order = (list(range(ns)) if fw
                                   else list(range(ns - 1, -1, -1)))
                        if b == 0 and layer == 0 and not fw:
                            bt["w1b"] = load_biases(b1b, "b1b")
                            load_w("w1b", w1b, k1)
                        btile = bt[wname]
                        carry = [cpool.tile([P, 1], F32, tag=f"c{hh}",
                                            name=f"carry{hh}")
                                 for hh in range(hc)]
                        if layer == 0:
                            off = 0 if fw else hc
                            for c in range(hc):
                                y1c[off + c] = y1pool.tile(
                                    [P, s], FP16, tag=f"y1c{off + c}",
                                    name=f"y1c{off + c}")
                        for si, s_idx in enumerate(s_order):
                            s0 = s_idx * s_tile
                            if layer == 0:
                                pos = l0_pos[(b, fw, s_idx)]
                                if pos + 1 < len(l0_iters):
                                    issue_l0_input(l0_iters[pos + 1])
                                t = in_tiles.pop((b, fw, s_idx))
                                ins = [t[:, k * s_tile : (k + 1) * s_tile]
                                       for k in range(k0)]
                            else:
                                ins = [y1c[k][:, s0 : s0 + s_tile]
                                       for k in range(k1)]
                            for hh in range(hc):
                                ps = [
                                    ppool.tile([P, s_tile], F32, tag=f"ps{g}",
                                               name=f"ps{g}",
                                               bufs=(3 if g < 2 else 2))
                                    for g in range(3)
                                ]
                                for g in range(3):
                                    cols = slice(g * h + hh * P,
                                                 g * h + (hh + 1) * P)
                                    for k in range(kch):
                                        nc.tensor.matmul(
                                            ps[g][:],
                                            wview(wname, k, cols),
                                            ins[k],
                                            start=(k == 0),
                                            stop=(k == kch - 1),
                                        )
                                zn = spool.tile([P, s_tile], F32, tag="zn",
                                                name="zn")
                                f_ = spool.tile([P, s_tile], F32, tag="f",
                                                name="f", bufs=4)
                                o = spool.tile([P, s_tile], F32, tag="o",
                                               name="o", bufs=6)
                                g_ = spool.tile([P, s_tile], F32, tag="g",
                                                name="g")
                                c_ = spool.tile([P, s_tile], F32, tag="c",
                                                name="c", bufs=6)
                                nc.scalar.activation(f_[:], ps[1][:],
                                                     ACT.Sigmoid,
                                                     bias=btile[1, hh][:])
                                # host negated the z-gate W/b: Tanh gives -z
                                nc.scalar.activation(zn[:], ps[0][:], ACT.Tanh,
                                                     bias=btile[0, hh][:])
                                nc.scalar.activation(o[:], ps[2][:],
                                                     ACT.Sigmoid,
                                                     bias=btile[2, hh][:])
                                # g = (f - 1) * (-z) = (1 - f) z, one DVE op
                                nc.vector.scalar_tensor_tensor(
                                    g_[:], f_[:], 1.0, zn[:],
                                    ALU.subtract, ALU.mult)
                                # c_t = f_t * c_prev + g_t (bw: reversed time)
                                if fw:
                                    sc = (c_[:], f_[:], g_[:])
                                    carry_col = slice(s_tile - 1, s_tile)
                                else:
                                    sc = (c_[:, ::-1], f_[:, ::-1], g_[:, ::-1])
                                    carry_col = slice(0, 1)
                                init = 0.0 if si == 0 else carry[hh][:]
                                nc.vector.tensor_tensor_scan(
                                    sc[0], sc[1], sc[2], init,
                                    ALU.mult, ALU.add)
                                if si < ns - 1:
                                    nc.gpsimd.tensor_copy(carry[hh][:],
                                                          c_[:, carry_col])
                                if layer == 0:
                                    chunk = y1c[(0 if fw else hc) + hh]
                                    nc.gpsimd.tensor_mul(
                                        chunk[:, s0 : s0 + s_tile],
                                        o[:], c_[:])
                                else:
                                    if hh == 0:
                                        ybig = spool.tile(
                                            [P, hc * s_tile], F32, tag="y",
                                            name="y", bufs=2)
                                    nc.gpsimd.tensor_mul(
                                        ybig[:, hh * s_tile : (hh + 1) * s_tile],
                                        o[:], c_[:])
                                    if hh == hc - 1:
                                        dst3 = out_t[
                                            b, dir_off : dir_off + h,
                                            s0 : s0 + s_tile].rearrange(
                                                "(a p) c -> p a c", p=P)
                                        nc.sync.dma_start(
                                            dst3,
                                            ybig[:].rearrange(
                                                "p (a c) -> p a c", a=hc))

    nc.finalize()
    return nc


_NC_CACHE = {}


def _get_nc():
    if "v3" not in _NC_CACHE:
        _NC_CACHE["v3"] = build_nc()
    return _NC_CACHE["v3"]


def kernel(X, seqlens, W_fw0, b_fw0, W_bw0, b_bw0, W_fw1, b_fw1, W_bw1, b_bw1,
           mm_dtype="fp16", trace=False):
    """Full-input entry point: shards over 8 cores, returns [B, S, 2H] f32."""
    del seqlens, mm_dtype  # seqlens unused by the reference computation
    X = np.asarray(X, dtype=np.float32)

    def neg_z(v, dt):
        """Negate the z-gate block so tanh(ps+b) = -z on device."""
        v = np.array(np.asarray(v, dtype=np.float32))
        v[..., :H] *= -1.0
        return np.ascontiguousarray(v.astype(dt))

    def btab(v):
        """[3H] bias -> [P, 3*H/P] column table (z-gate negated)."""
        v = neg_z(v, np.float32)  # [3H]
        t = v.reshape(3 * H // P, P).T  # [P, 3hc]
        return np.ascontiguousarray(t)

    weights = {
        "w0f": neg_z(W_fw0, np.float16), "b0f": btab(b_fw0),
        "w0b": neg_z(W_bw0, np.float16), "b0b": btab(b_bw0),
        "w1f": neg_z(W_fw1, np.float16), "b1f": btab(b_fw1),
        "w1b": neg_z(W_bw1, np.float16), "b1b": btab(b_bw1),
    }

    nc = _get_nc()
    in_maps = []
    for i in range(N_CORES):
        rows = X[i * BC : (i + 1) * BC]  # [BC, S, D]
        xt_i = np.ascontiguousarray(
            rows.transpose(0, 2, 1).astype(np.float16))  # [BC, D, S] fp16
        in_maps.append({"xt": xt_i, **weights})

    res = bass_utils.run_bass_kernel_spmd(
        nc, in_maps, core_ids=list(range(N_CORES)), trace=trace
    )
    out = np.empty((B, S, 2 * H), dtype=np.float32)
    for i in range(N_CORES):
        out_t = res.results[i]["out_t"]  # [BC, 2H, S]
        out[i * BC : (i + 1) * BC] = out_t.transpose(0, 2, 1)
    kernel.last_results = res
    return out


# revision 19
# speedup vs baseline: 1.0120x; 1.0026x over previous
"""Bass/Trainium2 kernel for a 2-layer bidirectional QRNN (fo-pooling).

Reference computation (per layer, per direction):
    ZFO = X @ W + b            # [S, B, 3H]
    Z, F, O = split(ZFO); Z = tanh(Z); F = sigmoid(F); O = sigmoid(O)
    c_t = F_t * c_{t-1} + (1 - F_t) * Z_t        (bw direction: reversed time)
    Y_dir = O * C
    Y = concat(Y_fw, Y_bw)     # [S, B, 2H]
Two stacked layers; output is [B, S, 2H].

Sharding: data-parallel over batch. B=16 rows -> 2 rows per NeuronCore x 8.
Each core runs both layers for its 2 rows; no collectives.

Design (evolved from a 635us fp32r/DRAM-roundtrip baseline; now ~525us,
PE-bound at ~96% of the 2304-matmul fp16 streaming floor):
- fp16 matmul operands, PRE-CAST ON THE HOST: X and all weights are fed
  to DRAM as float16, so matmul inputs DMA straight into SBUF with no
  on-chip casts or staging (fp16, unlike fp32r, is a real DMA-able
  dtype). Measured matmul: 216ns/[128x128x512] vs 233ns fp32r (the
  4-byte fp32r moving operand saturates SBUF read bandwidth). fp16
  quantization of X/W adds ~5e-4 rel err (gate: 2e-2).
- The inter-layer activation Y1 lives entirely in SBUF as fp16 (8 chunk
  tiles [128, S] per row), never touching DRAM. Rows are processed
  b-major (L0fw, L0bw, L1bw, L1fw per row) so only one row's Y1 is
  live. Layer 1 runs bw FIRST: its first input tile (s = S-tile) is
  finished at L0bw's first iteration, while L1fw's first input (s = 0)
  is only finished by L0bw's LAST iteration -- bw-first removes an
  ~8.6us/row PE stall at the layer boundary.
- Z-gate weight/bias columns are negated on the host, so the Tanh
  activation directly yields zneg = -z and the scan's g-input
  g = (1-f)*z = (f-1)*zneg is ONE fused DVE scalar_tensor_tensor
  (scalar_tensor_tensor is NOT supported on GpSimd). Biases are
  host-pretabled to [P, 3*H/P] so their DMA is contiguous (a
  rearranging gather DMA cost ~8us of descriptor processing).
- Post-PSUM values stay f32 (an fp16 scan measured SLOWER on DVE).
- Engine balance per layer-0 s-tile (PE: 10.4us): Scalar = 12
  activations (7.8); DVE = 4 scans + 4 fused g (9.6 incl slot waits);
  GpSimd = 4 y-mults + carry columns (6.5).
- DMA discipline: a DIRECT2D issue costs ~0.7us of SEQUENCER time, so
  (a) every multi-part transfer is batched into one 3-D-AP DMA (one per
  weight set, one per input s-tile, one per output s-tile), and (b) all
  DMAs ride the sync queue, keeping the Scalar sequencer free to decode
  activations (DMA issues queued ahead of acts once stalled the act
  stream ~10us and starved PSUM recycling). Input s-tile i+1 is
  prefetched at the start of s-tile i.
- Cold-start: 13 dummy matmuls on memset scratch hold the PE p-state at
  full clock through the ~5us DMA bring-up (cold matmuls run 2-2.7x
  slow), and two dummy activations force both act-table loads into the
  preamble window (an inline table switch costs 1.3us mid-pipeline).

The time recurrence uses the DVE `tensor_tensor_scan` instruction
(state = f*state + g along the free axis); the bw direction runs the scan
through reversed access patterns with s-tiles processed in descending
order, chaining the carry via a [128,1] column copy.
"""

import numpy as np

import concourse.bacc as bacc
import concourse.mybir as mybir
from concourse import bass_utils
from concourse.tile import TileContext

# problem dims (hardcoded per spec)
B, S, D, H = 16, 2048, 512, 512
N_CORES = 8
BC = B // N_CORES  # batch rows per core
P = 128  # SBUF partitions
S_TILE = 512

F32 = mybir.dt.float32
FP16 = mybir.dt.float16
ACT = mybir.ActivationFunctionType
ALU = mybir.AluOpType


def build_nc(bc=BC, s=S, d=D, h=H, s_tile=S_TILE):
    """Build the SPMD Bass program (same program on every core)."""
    nc = bacc.Bacc("TRN2", target_bir_lowering=False)

    xt = nc.dram_tensor("xt", [bc, d, s], FP16, kind="ExternalInput")
    w0f = nc.dram_tensor("w0f", [d, 3 * h], FP16, kind="ExternalInput")
    w0b = nc.dram_tensor("w0b", [d, 3 * h], FP16, kind="ExternalInput")
    b0f = nc.dram_tensor("b0f", [P, 3 * (h // P)], F32, kind="ExternalInput")
    b0b = nc.dram_tensor("b0b", [P, 3 * (h // P)], F32, kind="ExternalInput")
    w1f = nc.dram_tensor("w1f", [2 * h, 3 * h], FP16, kind="ExternalInput")
    w1b = nc.dram_tensor("w1b", [2 * h, 3 * h], FP16, kind="ExternalInput")
    b1f = nc.dram_tensor("b1f", [P, 3 * (h // P)], F32, kind="ExternalInput")
    b1b = nc.dram_tensor("b1b", [P, 3 * (h // P)], F32, kind="ExternalInput")
    out_t = nc.dram_tensor("out_t", [bc, 2 * h, s], F32, kind="ExternalOutput")

    ns = s // s_tile
    hc = h // P
    k0 = d // P       # layer-0 contraction chunks
    k1 = 2 * h // P   # layer-1 contraction chunks

    with TileContext(nc) as tc:
        with (
            tc.tile_pool(name="wpool", bufs=1) as wpool,     # fp16 weights
            tc.tile_pool(name="y1pool", bufs=1) as y1pool,   # inter-layer act
            tc.tile_pool(name="scr", bufs=3) as spool,       # zn/f/o/g/c/y
            tc.tile_pool(name="carry", bufs=1) as cpool,
            tc.tile_pool(name="instream", bufs=1) as ypool,  # layer-0 input
            tc.tile_pool(name="ps", bufs=1, space="PSUM") as ppool,
        ):
            # ---------------- weights ----------------
            # One [P, k*3h] tile and ONE 3-D-AP DMA per weight set: DIRECT2D
            # issues cost ~0.7us of sequencer time each, so batching matters.
            wk = {
                nm: wpool.tile([P, kc * 3 * h], FP16, tag=nm, name=nm)
                for nm, kc in (("w0f", k0), ("w0b", k0),
                               ("w1f", k1), ("w1b", k1))
            }

            def wview(name, k, cols):
                base = k * 3 * h
                return wk[name][:, base + cols.start : base + cols.stop]

            def load_w(name, wd, k_chunks):
                src3 = wd[:, :].rearrange("(a p) c -> p a c", p=P)
                dst3 = wk[name][:].rearrange("p (a c) -> p a c", a=k_chunks)
                nc.sync.dma_start(dst3, src3)

            def load_biases(bd, prefix):
                """One plain DMA of the host-pretabled [P, 3*hc] bias;
                returns per-(gate, h-chunk) [P, 1] views."""
                btab = wpool.tile([P, 3 * hc], F32, tag=f"{prefix}_btab",
                                  name=f"{prefix}_btab")
                nc.sync.dma_start(btab[:], bd[:, :])
                return {
                    (g, hh): btab[:, g * hc + hh : g * hc + hh + 1]
                    for g in range(3)
                    for hh in range(hc)
                }

            # PE warm-up: dummy matmuls on memset scratch keep the PE
            # busy through the DMA bring-up window, so the p-state governor
            # is at full clock when the first real matmul issues.
            warm_w = wpool.tile([P, P], FP16, tag="warm_w", name="warm_w")
            warm_m = wpool.tile([P, s_tile], FP16, tag="warm_m", name="warm_m")
            nc.gpsimd.memset(warm_w[:], 0.0)
            nc.gpsimd.memset(warm_m[:], 0.0)
            warm_ps = ppool.tile([P, s_tile], F32, tag="ps2", name="warm_ps",
                                 bufs=2)
            for _ in range(16):
                nc.tensor.matmul(warm_ps[:], warm_w[:], warm_m[:],
                                 start=True, stop=True)
            warm_act = wpool.tile([P, 1], F32, tag="warm_act", name="warm_act")
            nc.scalar.activation(warm_act[:], warm_w[:, 0:1], ACT.Sigmoid)
            nc.scalar.activation(warm_act[:], warm_w[:, 0:1], ACT.Tanh)

            bt = {}
            # Startup parallelism: w0f streams on the sync queue while the
            # first input s-tile + b0f bias ride the otherwise-idle scalar
            # queue (no activations decode before ~15us).
            load_w("w0f", w0f, k0)
            # ---------------- layer-0 input prefetch ----------------
            l0_iters = []
            for b in range(bc):
                for fw in (True, False):
                    order = range(ns) if fw else range(ns - 1, -1, -1)
                    for si in order:
                        l0_iters.append((b, fw, si))
            l0_pos = {key: i for i, key in enumerate(l0_iters)}
            in_tiles = {}

            def issue_l0_input(key):
                b, fw, si = key
                s0 = si * s_tile
                t = ypool.tile([P, k0 * s_tile], FP16, tag="inr", bufs=2,
                               name="inr")
                src3 = xt[b, :, s0 : s0 + s_tile].rearrange(
                    "(k p) s -> p k s", p=P)
                nc.sync.dma_start(
                    t[:].rearrange("p (k s) -> p k s", k=k0), src3)
                in_tiles[key] = t

            b0, fw0, si0 = l0_iters[0]
            t0_ = ypool.tile([P, k0 * s_tile], FP16, tag="inr", bufs=2,
                             name="inr")
            for k in range(k0):
                nc.scalar.dma_start(
                    t0_[:, k * s_tile : (k + 1) * s_tile],
                    xt[b0, k * P : (k + 1) * P,
                       si0 * s_tile : (si0 + 1) * s_tile])
            in_tiles[l0_iters[0]] = t0_
            btab0f = wpool.tile([P, 3 * hc], F32, tag="b0f_btab",
                                name="b0f_btab")
            nc.scalar.dma_start(btab0f[:], b0f[:, :])
            bt["w0f"] = {
                (g, hh): btab0f[:, g * hc + hh : g * hc + hh + 1]
                for g in range(3) for hh in range(hc)
            }
            # Remaining weights: queue now, they drain through slack long
            # before first use (w0b ~45us in, w1f ~90us in).
            bt["w0b"] = load_biases(b0b, "b0b")
            bt["w1f"] = load_biases(b1f, "b1f")
            load_w("w0b", w0b, k0)
            load_w("w1f", w1f, k1)

            y1c = {}  # chunk index 0..7 -> current row's SBUF tile

            for b in range(bc):
                for layer in (0, 1):
                    for fw in ((True, False) if layer == 0 else (False, True)):
                        wname = f"w{layer}{'f' if fw else 'b'}"
                        kch = k0 if layer == 0 else k1
                        dir_off = 0 if fw else h
                        s_order = (list(range(ns)) if fw
                                   else list(range(ns - 1, -1, -1)))
                        if b == 0 and layer == 0 and not fw:
                            bt["w1b"] = load_biases(b1b, "b1b")
                            load_w("w1b", w1b, k1)
                        btile = bt[wname]
                        carry = [cpool.tile([P, 1], F32, tag=f"c{hh}",
                                            name=f"carry{hh}")
                                 for hh in range(hc)]
                        if layer == 0:
                            off = 0 if fw else hc
                            for c in range(hc):
                                y1c[off + c] = y1pool.tile(
                                    [P, s], FP16, tag=f"y1c{off + c}",
                                    name=f"y1c{off + c}")
                        for si, s_idx in enumerate(s_order):
                            s0 = s_idx * s_tile
                            if layer == 0:
                                pos = l0_pos[(b, fw, s_idx)]
                                if pos + 1 < len(l0_iters):
                                    issue_l0_input(l0_iters[pos + 1])
                                t = in_tiles.pop((b, fw, s_idx))
                                ins = [t[:, k * s_tile : (k + 1) * s_tile]
                                       for k in range(k0)]
                            else:
                                ins = [y1c[k][:, s0 : s0 + s_tile]
                                       for k in range(k1)]
                            for hh in range(hc):
                                ps = [
                                    ppool.tile([P, s_tile], F32, tag=f"ps{g}",
                                               name=f"ps{g}",
                                               bufs=(3 if g < 2 else 2))
                                    for g in range(3)
                                ]
                                for g in range(3):
                                    cols = slice(g * h + hh * P,
                                                 g * h + (hh + 1) * P)
                                    for k in range(kch):
                                        nc.tensor.matmul(
                                            ps[g][:],
                                            wview(wname, k, cols),
                                            ins[k],
                                            start=(k == 0),
                                            stop=(k == kch - 1),
                                        )
                                zn = spool.tile([P, s_tile], F32, tag="zn",
                                                name="zn")
                                f_ = spool.tile([P, s_tile], F32, tag="f",
                                                name="f", bufs=4)
                                o = spool.tile([P, s_tile], F32, tag="o",
                                               name="o", bufs=6)
                                g_ = spool.tile([P, s_tile], F32, tag="g",
                                                name="g")
                                c_ = spool.tile([P, s_tile], F32, tag="c",
                                                name="c", bufs=6)
                                nc.scalar.activation(f_[:], ps[1][:],
                                                     ACT.Sigmoid,
                                                     bias=btile[1, hh][:])
                                # host negated the z-gate W/b: Tanh gives -z
                                nc.scalar.activation(zn[:], ps[0][:], ACT.Tanh,
                                                     bias=btile[0, hh][:])
                                nc.scalar.activation(o[:], ps[2][:],
                                                     ACT.Sigmoid,
                                                     bias=btile[2, hh][:])
                                # g = (f - 1) * (-z) = (1 - f) z, one DVE op
                                nc.vector.scalar_tensor_tensor(
                                    g_[:], f_[:], 1.0, zn[:],
                                    ALU.subtract, ALU.mult)
                                # c_t = f_t * c_prev + g_t (bw: reversed time)
                                if fw:
                                    sc = (c_[:], f_[:], g_[:])
                                    carry_col = slice(s_tile - 1, s_tile)
                                else:
                                    sc = (c_[:, ::-1], f_[:, ::-1], g_[:, ::-1])
                                    carry_col = slice(0, 1)
                                init = 0.0 if si == 0 else carry[hh][:]
                                nc.vector.tensor_tensor_scan(
                                    sc[0], sc[1], sc[2], init,
                                    ALU.mult, ALU.add)
                                if si < ns - 1:
                                    nc.gpsimd.tensor_copy(carry[hh][:],
                                                          c_[:, carry_col])
                                if layer == 0:
                                    chunk = y1c[(0 if fw else hc) + hh]
                                    nc.gpsimd.tensor_mul(
                                        chunk[:, s0 : s0 + s_tile],
                                        o[:], c_[:])
                                else:
                                    if hh == 0:
                                        ybig = spool.tile(
                                            [P, hc * s_tile], F32, tag="y",
                                            name="y", bufs=2)
                                    nc.gpsimd.tensor_mul(
                                        ybig[:, hh * s_tile : (hh + 1) * s_tile],
                                        o[:], c_[:])
                                    if hh == hc - 1:
                                        dst3 = out_t[
                                            b, dir_off : dir_off + h,
                                            s0 : s0 + s_tile].rearrange(
                                                "(a p) c -> p a c", p=P)
                                        nc.sync.dma_start(
                                            dst3,
                                            ybig[:].rearrange(
                                                "p (a c) -> p a c", a=hc))

    nc.finalize()
    return nc


_NC_CACHE = {}


def _get_nc():
    if "v3" not in _NC_CACHE:
        _NC_CACHE["v3"] = build_nc()
    return _NC_CACHE["v3"]


def kernel(X, seqlens, W_fw0, b_fw0, W_bw0, b_bw0, W_fw1, b_fw1, W_bw1, b_bw1,
           mm_dtype="fp16", trace=False):
    """Full-input entry point: shards over 8 cores, returns [B, S, 2H] f32."""
    del seqlens, mm_dtype  # seqlens unused by the reference computation
    X = np.asarray(X, dtype=np.float32)

    def neg_z(v, dt):
        """Negate the z-gate block so tanh(ps+b) = -z on device."""
        v = np.array(np.asarray(v, dtype=np.float32))
        v[..., :H] *= -1.0
        return np.ascontiguousarray(v.astype(dt))

    def btab(v):
        """[3H] bias -> [P, 3*H/P] column table (z-gate negated)."""
        v = neg_z(v, np.float32)  # [3H]
        t = v.reshape(3 * H // P, P).T  # [P, 3hc]
        return np.ascontiguousarray(t)

    weights = {
        "w0f": neg_z(W_fw0, np.float16), "b0f": btab(b_fw0),
        "w0b": neg_z(W_bw0, np.float16), "b0b": btab(b_bw0),
        "w1f": neg_z(W_fw1, np.float16), "b1f": btab(b_fw1),
        "w1b": neg_z(W_bw1, np.float16), "b1b": btab(b_bw1),
    }

    nc = _get_nc()
    in_maps = []
    for i in range(N_CORES):
        rows = X[i * BC : (i + 1) * BC]  # [BC, S, D]
        xt_i = np.ascontiguousarray(
            rows.transpose(0, 2, 1).astype(np.float16))  # [BC, D, S] fp16
        in_maps.append({"xt": xt_i, **weights})

    res = bass_utils.run_bass_kernel_spmd(
        nc, in_maps, core_ids=list(range(N_CORES)), trace=trace
    )
    out = np.empty((B, S, 2 * H), dtype=np.float32)
    for i in range(N_CORES):
        out_t = res.results[i]["out_t"]  # [BC, 2H, S]
        out[i * BC : (i + 1) * BC] = out_t.transpose(0, 2, 1)
    kernel.last_results = res
    return out
